# revision 6
# baseline (speedup 1.0000x reference)
"""MoE (MiMoV2 FlashMoE) Trainium2 kernel: expert-parallel over 8 NeuronCores.

Strategy:
  Phase 1 (device): router — logits = x @ w_router.T computed in exact fp32
    (selection must match the reference bit-for-bit; any top-4 flip costs
    ~14% output error), top-4 via iterative max/mask, combine weights =
    sigmoid(logit) normalized over the selected 4. Each core handles
    T/8 = 512 tokens. Output: dense combine matrix [T, E].
  Host: compaction — per-expert token index lists (data movement only),
    load-balanced expert->(core,slot) assignment: experts sorted by load
    descending, slot j takes ranks [8j, 8j+8) one per core, slot capacity
    = max load in the slot (rounded up to 32). This cuts padded columns
    ~16% vs a global fixed capacity.
  Phase 2 (device): experts — one expert per (core, slot). All matmuls in
    bf16 (same 1 cycle/row PE rate as f32r on TRN2, half the HBM/SBUF
    traffic; output rel err ~3e-3 vs the 2e-2 budget). G = Wg x, U = Wu x
    accumulate in fp32 PSUM; H = silu(G)*U*combine in fp32, cast to bf16;
    Y = Wd H in fp32 out.
  Host: scatter-add per-expert outputs into y [T, H].
"""
import math
import numpy as np
import ml_dtypes
from contextlib import ExitStack

import concourse.bass as bass
import concourse.mybir as mybir
import concourse.tile as tile
from concourse import bacc
from concourse.bass_utils import run_bass_kernel_spmd

F32 = mybir.dt.float32
F32R = mybir.dt.float32r
BF16 = mybir.dt.bfloat16
NPBF16 = ml_dtypes.bfloat16

# Problem shapes (hardcoded per contract)
E = 32          # experts
TOPK = 4
H = 1024        # hidden
I = 768         # intermediate
B, S = 2, 2048
T = B * S       # 4096 tokens
NCORES = 8
EPC = E // NCORES    # expert slots per core = 4
TPC = T // NCORES    # router tokens per core = 512
KH = H // 128        # 8 contraction chunks over H
KI = I // 128        # 6 contraction chunks over I

_program_cache = {}


def _ctiles(C):
    """Split C into near-equal tiles, each <= 512 (PSUM bank)."""
    n = max(1, math.ceil(C / 512))
    base = C // n
    rem = C - base * n
    sizes = [base + (1 if i < rem else 0) for i in range(n)]
    out, off = [], 0
    for s in sizes:
        out.append((off, s))
        off += s
    return out


def build_router(reps=1):
    """Per-core: logits computed token-major directly — per 128-token tile,
    stationary = x chunk [128(H), 128(tok)], moving = w_router chunk
    [128(H), E], accumulating logits [128(tok), E] over 8 H-chunks in PSUM.
    Contraction stays on the partition dim, so the fp32 numerics are
    identical to the transposed layout, with no DVE transposes. Then a
    batched top-4 + combine-weight computation on a single [128, 4, E]
    tile. Selection compares exact fp32 logits."""
    nc = bacc.Bacc()
    NT = TPC // 128  # 4 token tiles
    xTc = nc.dram_tensor("xTc", [H, TPC], F32, kind="ExternalInput")
    wrT = nc.dram_tensor("wrT", [H, E], F32, kind="ExternalInput")
    comb_out = nc.dram_tensor("comb", [NT, 128, E], F32, kind="ExternalOutput")
    with ExitStack() as ctx:
        tc = ctx.enter_context(tile.TileContext(nc))
        sb = ctx.enter_context(tc.tile_pool(name="sb", bufs=1))
        work = ctx.enter_context(tc.tile_pool(name="work", bufs=2))
        ps = ctx.enter_context(tc.tile_pool(name="ps", bufs=2, space="PSUM"))

        xr = sb.tile([128, KH, TPC], F32)
        wr = sb.tile([128, KH, E], F32)
        for k in range(KH):
            nc.sync.dma_start(out=xr[:, k, :], in_=xTc[k * 128:(k + 1) * 128, :])
            nc.sync.dma_start(out=wr[:, k, :], in_=wrT[k * 128:(k + 1) * 128, :])

        for _ in range(reps):
            lt_p = ps.tile([128, NT, E], F32)   # logits, token-major
            for t in range(NT):
                for k in range(KH):
                    nc.tensor.matmul(lt_p[:, t, :],
                                     xr[:, k, t * 128:(t + 1) * 128],
                                     wr[:, k, :],
                                     start=(k == 0), stop=(k == KH - 1))
            lt = work.tile([128, NT, E], F32)
            nc.vector.tensor_copy(lt, lt_p)
            # batched top-4: find 4th max per token via iterative masking
            cur = work.tile([128, NT, E], F32)
            nc.vector.tensor_copy(cur, lt)
            m = work.tile([128, NT, 1], F32)
            ge = work.tile([128, NT, E], F32)
            for _k in range(TOPK - 1):
                nc.vector.reduce_max(m, cur, axis=mybir.AxisListType.X)
                nc.vector.tensor_tensor(ge, cur, m.broadcast_to((128, NT, E)),
                                        op=mybir.AluOpType.is_ge)
                nc.vector.scalar_tensor_tensor(cur, ge, -1e30, cur,
                                               op0=mybir.AluOpType.mult,
                                               op1=mybir.AluOpType.add)
            nc.vector.reduce_max(m, cur, axis=mybir.AxisListType.X)
            # sel = (logits >= 4th max), combine = sel*sigmoid normalized
            sel = work.tile([128, NT, E], F32)
            nc.vector.tensor_tensor(sel, lt, m.broadcast_to((128, NT, E)),
                                    op=mybir.AluOpType.is_ge)
            sig = work.tile([128, NT, E], F32)
            nc.scalar.activation(sig, lt, mybir.ActivationFunctionType.Sigmoid)
            wsel = work.tile([128, NT, E], F32)
            nc.vector.tensor_mul(wsel, sel, sig)
            ssum = work.tile([128, NT, 1], F32)
            nc.vector.reduce_sum(ssum, wsel, axis=mybir.AxisListType.X)
            nc.vector.tensor_scalar_add(ssum, ssum, 1e-20)
            rsum = work.tile([128, NT, 1], F32)
            nc.vector.reciprocal(rsum, ssum)
            ct = work.tile([128, NT, E], F32)
            nc.vector.tensor_tensor(ct, wsel, rsum.broadcast_to((128, NT, E)),
                                    op=mybir.AluOpType.mult)
            for t in range(NT):
                nc.sync.dma_start(out=comb_out[t], in_=ct[:, t, :])
    nc.finalize()
    return nc


def build_experts(caps, reps=1):
    """Expert MLP kernel, one expert per slot j with capacity caps[j].
    Per-core inputs (pre-laid-out for SBUF tiles, all bf16 except cw):
      xg{j} [128, KH, caps[j]]        bf16  xg[p,k,c] = x[tok_c, k*128+p]
      wgu   [EPC, KI, 2, 128, KH, 128] bf16 [...,0,i]=w_gate[e,m*128+i,k*128+p]
      wd    [EPC, KH, 128, KI, 128]   bf16  wd[j,h,p,k,o]=w_down[e,h*128+o,k*128+p]
      cw{j} [1, caps[j]]              f32   combine weights (0 on padding)
    Output: yg{j} [128, KH, caps[j]] f32, yg[p,h,c] = y^T[h*128+p, c]
    (combine-weighted, transposed)."""
    caps = tuple(caps)
    nc = bacc.Bacc()
    xg = [nc.dram_tensor(f"xg{j}", [128, KH, caps[j]], BF16,
                         kind="ExternalInput") for j in range(EPC)]
    wgu = nc.dram_tensor("wgu", [EPC, KI, 2, 128, KH, 128], BF16,
                         kind="ExternalInput")
    wd = nc.dram_tensor("wd", [EPC, KH, 128, KI, 128], BF16,
                        kind="ExternalInput")
    cw = [nc.dram_tensor(f"cw{j}", [1, caps[j]], F32, kind="ExternalInput")
          for j in range(EPC)]
    yg = [nc.dram_tensor(f"yg{j}", [128, KH, caps[j]], F32,
                         kind="ExternalOutput") for j in range(EPC)]
    warm_out = nc.dram_tensor("warm", [128, 1], F32, kind="ExternalOutput")

    with ExitStack() as ctx:
        tc = ctx.enter_context(tile.TileContext(nc))
        cwp = ctx.enter_context(tc.tile_pool(name="cwp", bufs=1))
        xgp = ctx.enter_context(tc.tile_pool(name="xgp", bufs=2))
        wgup = ctx.enter_context(tc.tile_pool(name="wgup", bufs=6))
        wdp = ctx.enter_context(tc.tile_pool(name="wdp", bufs=4))
        hp = ctx.enter_context(tc.tile_pool(name="hp", bufs=2))
        msc = ctx.enter_context(tc.tile_pool(name="msc", bufs=4))
        outp = ctx.enter_context(tc.tile_pool(name="outp", bufs=2))
        ps_gu = ctx.enter_context(tc.tile_pool(name="ps_gu", bufs=2, space="PSUM"))
        ps_d = ctx.enter_context(tc.tile_pool(name="ps_d", bufs=2, space="PSUM"))

        cwb = []
        for j in range(EPC):
            cwt = cwp.tile([128, caps[j]], F32, tag=f"cw{j}")
            nc.gpsimd.dma_start(out=cwt,
                                in_=cw[j][0:1, :].partition_broadcast(128))
            cwb.append(cwt)

        # PE warm-up: keep TensorE busy while the first weight/activation
        # DMAs land, so the HAM clock-gate releases (1.2 -> 2.4 GHz) before
        # real matmuls start. Results are dumped to a debug output.
        wtile = cwp.tile([128, 512], F32R, tag="warm")
        nc.vector.memset(wtile.bitcast(F32), 0.0)
        wps = ps_d.tile([128, 512], F32, tag="warmp")
        for wi in range(6):
            nc.tensor.matmul(wps, wtile[:, :128], wtile,
                             start=(wi == 0), stop=(wi == 5))
        wres = cwp.tile([128, 1], F32, tag="warmres")
        nc.vector.tensor_copy(wres, wps[:, 0:1])
        nc.gpsimd.dma_start(out=warm_out[:], in_=wres)

        for _ in range(reps):
            for j in range(EPC):
                cap = caps[j]
                cts = _ctiles(cap)
                xg_t = xgp.tile([128, KH, cap], BF16)
                for k in range(KH):
                    eng = nc.sync if k % 2 == 0 else nc.scalar
                    eng.dma_start(out=xg_t[:, k, :], in_=xg[j][:, k, :])
                h_t = hp.tile([128, KI, cap], BF16)
                for m in range(KI):
                    wgu_t = wgup.tile([128, 2, KH, 128], BF16)
                    nc.sync.dma_start(out=wgu_t[:, 0], in_=wgu[j, m, 0])
                    nc.scalar.dma_start(out=wgu_t[:, 1], in_=wgu[j, m, 1])
                    for (c0, cn) in cts:
                        gp = ps_gu.tile([128, cn], F32, tag="gp")
                        for k in range(KH):
                            nc.tensor.matmul(gp, wgu_t[:, 0, k, :],
                                             xg_t[:, k, c0:c0 + cn],
                                             start=(k == 0), stop=(k == KH - 1))
                        up = ps_gu.tile([128, cn], F32, tag="up")
                        for k in range(KH):
                            nc.tensor.matmul(up, wgu_t[:, 1, k, :],
                                             xg_t[:, k, c0:c0 + cn],
                                             start=(k == 0), stop=(k == KH - 1))
                        sg = msc.tile([128, cn], F32, tag="sg")
                        nc.scalar.activation(sg, gp,
                                             mybir.ActivationFunctionType.Silu)
                        t1 = msc.tile([128, cn], F32, tag="t1")
                        nc.vector.tensor_mul(t1, sg, up)
                        nc.vector.tensor_mul(h_t[:, m, c0:c0 + cn], t1,
                                             cwb[j][:, c0:c0 + cn])
                yo_all = outp.tile([128, KH, cap], F32, tag="yo")
                for h in range(KH):
                    wd_t = wdp.tile([128, KI, 128], BF16)
                    eng = nc.sync if h % 2 == 0 else nc.scalar
                    eng.dma_start(out=wd_t, in_=wd[j, h])
                    for (c0, cn) in cts:
                        yp = ps_d.tile([128, cn], F32, tag="yp")
                        for k in range(KI):
                            nc.tensor.matmul(yp, wd_t[:, k, :],
                                             h_t[:, k, c0:c0 + cn],
                                             start=(k == 0), stop=(k == KI - 1))
                        nc.vector.tensor_copy(yo_all[:, h, c0:c0 + cn], yp)
                    if h % 2 == 1:
                        # stream out in 2-h chunks as they complete (SWDGE,
                        # keeping both HWDGE engines free for input reads)
                        nc.gpsimd.dma_start(out=yg[j][:, h - 1:h + 1, :],
                                            in_=yo_all[:, h - 1:h + 1, :])
    nc.finalize()
    return nc


def _get_router():
    if "router" not in _program_cache:
        _program_cache["router"] = build_router()
    return _program_cache["router"]


def _get_experts(caps):
    key = ("experts", tuple(caps))
    if key not in _program_cache:
        _program_cache[key] = build_experts(caps)
    return _program_cache[key]


def prep_router_inputs(x):
    xT = np.ascontiguousarray(x.T)
    return xT


def plan_experts(combine):
    """Load-balanced assignment: experts sorted by load descending; slot j
    takes ranks [8j, 8j+8), one per core; cap_j = max load in slot j
    (rounded up to 32). Returns per-expert index lists, assignment
    (core, slot) -> expert, and slot capacities."""
    loads = (combine > 0).sum(axis=0).astype(np.int64)
    order = np.argsort(-loads, kind="stable")
    assign = [[int(order[8 * j + c]) for j in range(EPC)]
              for c in range(NCORES)]
    caps = tuple(int(max(64, math.ceil(loads[order[8 * j]] / 8) * 8))
                 for j in range(EPC))
    idx = [np.nonzero(combine[:, e])[0] for e in range(E)]
    return idx, assign, caps


def prep_expert_weights(w_gate, w_up, w_down):
    """Per-expert bf16 tile-exact layouts (done once per kernel() call)."""
    gb = w_gate.astype(NPBF16).reshape(E, KI, 128, KH, 128)
    ub = w_up.astype(NPBF16).reshape(E, KI, 128, KH, 128)
    db = w_down.astype(NPBF16).reshape(E, KH, 128, KI, 128)
    gb = np.ascontiguousarray(gb.transpose(0, 1, 4, 3, 2))  # (e, m, p, k, i)
    ub = np.ascontiguousarray(ub.transpose(0, 1, 4, 3, 2))
    db = np.ascontiguousarray(db.transpose(0, 1, 4, 3, 2))  # (e, h, p, k, o)
    return gb, ub, db


def prep_expert_inputs(x, combine, idx, assign, caps, gb, ub, db):
    """Build per-core in_maps with tile-exact layouts (contiguous DMA)."""
    xb = x.astype(NPBF16)
    in_maps = []
    for c in range(NCORES):
        m = {}
        wgu = np.empty((EPC, KI, 2, 128, KH, 128), NPBF16)
        wdh = np.empty((EPC, KH, 128, KI, 128), NPBF16)
        for j in range(EPC):
            e = assign[c][j]
            ii = idx[e]
            n = len(ii)
            cap = caps[j]
            xgj = np.zeros((128, KH, cap), NPBF16)
            cwj = np.zeros((1, cap), np.float32)
            if n:
                # [n, H] -> [n, KH, 128] -> [128, KH, n]
                xgj[:, :, :n] = xb[ii].reshape(n, KH, 128).transpose(2, 1, 0)
                cwj[0, :n] = combine[ii, e]
            m[f"xg{j}"] = xgj
            m[f"cw{j}"] = cwj
            wgu[j, :, 0] = gb[e]
            wgu[j, :, 1] = ub[e]
            wdh[j] = db[e]
        m["wgu"] = wgu
        m["wd"] = wdh
        in_maps.append(m)
    return in_maps


def kernel(hidden_states, w_router, w_gate, w_up, w_down):
    x = np.ascontiguousarray(np.asarray(hidden_states, np.float32)).reshape(T, H)
    w_gate = np.asarray(w_gate, np.float32)
    w_up = np.asarray(w_up, np.float32)
    w_down = np.asarray(w_down, np.float32)
    xT = prep_router_inputs(x)
    wrT = np.ascontiguousarray(np.asarray(w_router, np.float32).T)   # [H, E]

    # ---- Phase 1: router on device ----
    nc1 = _get_router()
    in_maps1 = [
        {"xTc": np.ascontiguousarray(xT[:, c * TPC:(c + 1) * TPC]), "wrT": wrT}
        for c in range(NCORES)
    ]
    r1 = run_bass_kernel_spmd(nc1, in_maps1, list(range(NCORES)))
    combine = np.concatenate(
        [r1.results[c]["comb"].reshape(TPC, E) for c in range(NCORES)], axis=0)

    # ---- Host: compaction (data movement only) ----
    idx, assign, caps = plan_experts(combine)
    gb, ub, db = prep_expert_weights(w_gate, w_up, w_down)
    in_maps2 = prep_expert_inputs(x, combine, idx, assign, caps, gb, ub, db)

    # ---- Phase 2: expert MLPs on device ----
    nc2 = _get_experts(caps)
    r2 = run_bass_kernel_spmd(nc2, in_maps2, list(range(NCORES)))

    # ---- Host: scatter-add (unique indices per expert) ----
    y = np.zeros((T, H), np.float32)
    for c in range(NCORES):
        for j in range(EPC):
            e = assign[c][j]
            ii = idx[e]
            n = len(ii)
            if n:
                # [128(p), KH(h), cap] -> [H, cap]: H index = h*128 + p
                ygj = r2.results[c][f"yg{j}"]
                yt = ygj.transpose(1, 0, 2).reshape(H, caps[j])
                y[ii] += yt[:, :n].T
    return y.reshape(B, S, H)


# revision 24
# speedup vs baseline: 1.1423x; 1.1423x over previous
"""MoE (MiMoV2 FlashMoE) Trainium2 kernel: expert-parallel over 8 NeuronCores.

Strategy:
  Phase 1 (device): router — logits = x @ w_router.T computed in exact fp32
    (selection must match the reference bit-for-bit; any top-4 flip costs
    ~14% output error), top-4 via iterative max/mask, combine weights =
    sigmoid(logit) normalized over the selected 4. Each core handles
    T/8 = 512 tokens. Output: dense combine matrix [T, E].
  Host: compaction — per-expert token index lists (data movement only),
    load-balanced expert->(core,slot) assignment: experts sorted by load
    descending, slot j takes ranks [8j, 8j+8) one per core, slot capacity
    = max load in the slot (rounded up to 32). This cuts padded columns
    ~16% vs a global fixed capacity.
  Phase 2 (device): experts — one expert per (core, slot). All matmuls in
    bf16 (same 1 cycle/row PE rate as f32r on TRN2, half the HBM/SBUF
    traffic; output rel err ~3e-3 vs the 2e-2 budget). G = Wg x, U = Wu x
    accumulate in fp32 PSUM; H = silu(G)*U*combine in fp32, cast to bf16;
    Y = Wd H in fp32 out.
  Host: scatter-add per-expert outputs into y [T, H].
"""
import math
import numpy as np
import ml_dtypes
from contextlib import ExitStack

import concourse.bass as bass
import concourse.mybir as mybir
import concourse.tile as tile
from concourse import bacc
from concourse.bass_utils import run_bass_kernel_spmd

F32 = mybir.dt.float32
F32R = mybir.dt.float32r
BF16 = mybir.dt.bfloat16
NPBF16 = ml_dtypes.bfloat16

# Problem shapes (hardcoded per contract)
E = 32          # experts
TOPK = 4
H = 1024        # hidden
I = 768         # intermediate
B, S = 2, 2048
T = B * S       # 4096 tokens
NCORES = 8
EPC = E // NCORES    # expert slots per core = 4
TPC = T // NCORES    # router tokens per core = 512
KH = H // 128        # 8 contraction chunks over H
KI = I // 128        # 6 contraction chunks over I

_program_cache = {}


def _ctiles(C):
    """Split C into near-equal tiles, each <= 512 (PSUM bank)."""
    n = max(1, math.ceil(C / 512))
    base = C // n
    rem = C - base * n
    sizes = [base + (1 if i < rem else 0) for i in range(n)]
    out, off = [], 0
    for s in sizes:
        out.append((off, s))
        off += s
    return out


def build_router(reps=1):
    """Per-core: logits computed token-major directly — per 128-token tile,
    stationary = x chunk [128(H), 128(tok)], moving = w_router chunk
    [128(H), E], accumulating logits [128(tok), E] over 8 H-chunks in PSUM.
    Contraction stays on the partition dim, so the fp32 numerics are
    identical to the transposed layout, with no DVE transposes. Then a
    batched top-4 + combine-weight computation on a single [128, 4, E]
    tile. Selection compares exact fp32 logits."""
    nc = bacc.Bacc()
    NT = TPC // 128  # 4 token tiles
    xTc = nc.dram_tensor("xTc", [H, TPC], F32, kind="ExternalInput")
    wrT = nc.dram_tensor("wrT", [H, E], F32, kind="ExternalInput")
    comb_out = nc.dram_tensor("comb", [NT, 128, E], F32, kind="ExternalOutput")
    with ExitStack() as ctx:
        tc = ctx.enter_context(tile.TileContext(nc))
        sb = ctx.enter_context(tc.tile_pool(name="sb", bufs=1))
        work = ctx.enter_context(tc.tile_pool(name="work", bufs=2))
        ps = ctx.enter_context(tc.tile_pool(name="ps", bufs=2, space="PSUM"))

        xr = sb.tile([128, KH, TPC], F32)
        wr = sb.tile([128, KH, E], F32)
        for k in range(KH):
            nc.sync.dma_start(out=xr[:, k, :], in_=xTc[k * 128:(k + 1) * 128, :])
            nc.sync.dma_start(out=wr[:, k, :], in_=wrT[k * 128:(k + 1) * 128, :])

        for _ in range(reps):
            lt_p = ps.tile([128, NT, E], F32)   # logits, token-major
            for t in range(NT):
                for k in range(KH):
                    nc.tensor.matmul(lt_p[:, t, :],
                                     xr[:, k, t * 128:(t + 1) * 128],
                                     wr[:, k, :],
                                     start=(k == 0), stop=(k == KH - 1))
            lt = work.tile([128, NT, E], F32)
            nc.vector.tensor_copy(lt, lt_p)
            # batched top-4: find 4th max per token via iterative masking
            cur = work.tile([128, NT, E], F32)
            nc.vector.tensor_copy(cur, lt)
            m = work.tile([128, NT, 1], F32)
            ge = work.tile([128, NT, E], F32)
            for _k in range(TOPK - 1):
                nc.vector.reduce_max(m, cur, axis=mybir.AxisListType.X)
                nc.vector.tensor_tensor(ge, cur, m.broadcast_to((128, NT, E)),
                                        op=mybir.AluOpType.is_ge)
                nc.vector.scalar_tensor_tensor(cur, ge, -1e30, cur,
                                               op0=mybir.AluOpType.mult,
                                               op1=mybir.AluOpType.add)
            nc.vector.reduce_max(m, cur, axis=mybir.AxisListType.X)
            # sel = (logits >= 4th max), combine = sel*sigmoid normalized
            sel = work.tile([128, NT, E], F32)
            nc.vector.tensor_tensor(sel, lt, m.broadcast_to((128, NT, E)),
                                    op=mybir.AluOpType.is_ge)
            sig = work.tile([128, NT, E], F32)
            nc.scalar.activation(sig, lt, mybir.ActivationFunctionType.Sigmoid)
            wsel = work.tile([128, NT, E], F32)
            nc.vector.tensor_mul(wsel, sel, sig)
            ssum = work.tile([128, NT, 1], F32)
            nc.vector.reduce_sum(ssum, wsel, axis=mybir.AxisListType.X)
            nc.vector.tensor_scalar_add(ssum, ssum, 1e-20)
            rsum = work.tile([128, NT, 1], F32)
            nc.vector.reciprocal(rsum, ssum)
            ct = work.tile([128, NT, E], F32)
            nc.vector.tensor_tensor(ct, wsel, rsum.broadcast_to((128, NT, E)),
                                    op=mybir.AluOpType.mult)
            for t in range(NT):
                nc.sync.dma_start(out=comb_out[t], in_=ct[:, t, :])
    nc.finalize()
    return nc


def build_experts(caps, reps=1):
    """Expert MLP kernel, one expert per slot j with capacity caps[j].
    Per-core inputs (pre-laid-out for SBUF tiles, all bf16 except cw).
    Weight tensors are partition-major so each expert's weights move in
    ONE long-row DMA (128 descriptors) instead of 12/8 short-row DMAs —
    DMA-issue SEQ time on the HWDGE queues is proportional to descriptor
    count and was the limiting resource:
      xg{j} [128, KH, caps[j]]          bf16  xg[p,k,c] = x[tok_c, k*128+p]
      wgu   [EPC, 128, KI, 2, KH, 128]  bf16  [j,p,m,0,k,i]=w_gate[e,m*128+i,k*128+p]
      wd    [EPC, 128, KH, KI, 128]     bf16  wd[j,p,h,k,o]=w_down[e,h*128+o,k*128+p]
      cw{j} [1, caps[j]]                f32   combine weights (0 on padding)
    Output: yg{j} [128, KH, caps[j]] f32, yg[p,h,c] = y^T[h*128+p, c]
    (combine-weighted, transposed)."""
    caps = tuple(caps)
    nc = bacc.Bacc()
    xg = [nc.dram_tensor(f"xg{j}", [128, KH, caps[j]], BF16,
                         kind="ExternalInput") for j in range(EPC)]
    wgu = nc.dram_tensor("wgu", [EPC, 128, KI, 2, KH, 128], BF16,
                         kind="ExternalInput")
    wd = nc.dram_tensor("wd", [EPC, 128, KH, KI, 128], BF16,
                        kind="ExternalInput")
    cw = [nc.dram_tensor(f"cw{j}", [1, caps[j]], F32, kind="ExternalInput")
          for j in range(EPC)]
    yg = [nc.dram_tensor(f"yg{j}", [128, KH, caps[j]], F32,
                         kind="ExternalOutput") for j in range(EPC)]
    warm_out = nc.dram_tensor("warm", [128, 1], F32, kind="ExternalOutput")

    with ExitStack() as ctx:
        tc = ctx.enter_context(tile.TileContext(nc))
        cwp = ctx.enter_context(tc.tile_pool(name="cwp", bufs=1))
        xgp = ctx.enter_context(tc.tile_pool(name="xgp", bufs=2))
        wgup = ctx.enter_context(tc.tile_pool(name="wgup", bufs=2))
        wdp = ctx.enter_context(tc.tile_pool(name="wdp", bufs=2))
        hp = ctx.enter_context(tc.tile_pool(name="hp", bufs=2))
        msc = ctx.enter_context(tc.tile_pool(name="msc", bufs=4))
        outp = ctx.enter_context(tc.tile_pool(name="outp", bufs=2))
        ps_gu = ctx.enter_context(tc.tile_pool(name="ps_gu", bufs=2, space="PSUM"))
        ps_d = ctx.enter_context(tc.tile_pool(name="ps_d", bufs=2, space="PSUM"))

        # PE warm-up: keep TensorE busy while the first weight/activation
        # DMAs land, so the HAM clock-gate releases (1.2 -> 2.4 GHz) before
        # real matmuls start. Results are dumped to a debug output. Issued
        # before everything else so nothing delays the first warm matmul.
        wtile = cwp.tile([128, 512], F32R, tag="warm")
        nc.gpsimd.memset(wtile.bitcast(F32), 0.0)
        wps = ps_d.tile([128, 512], F32, tag="warmp")
        NWARM = 20
        for wi in range(NWARM):
            nc.tensor.matmul(wps, wtile[:, :128], wtile,
                             start=(wi == 0), stop=(wi == NWARM - 1))

        cwb = []
        for j in range(EPC):
            cwt = cwp.tile([128, caps[j]], F32, tag=f"cw{j}")
            nc.gpsimd.dma_start(out=cwt,
                                in_=cw[j][0:1, :].partition_broadcast(128))
            cwb.append(cwt)

        # warm-out readback has no deadline; issue it after the cw loads
        wres = cwp.tile([128, 1], F32, tag="warmres")
        nc.vector.tensor_copy(wres, wps[:, 0:1])
        nc.gpsimd.dma_start(out=warm_out[:], in_=wres)

        def issue_expert(j):
            """Enqueue ALL of expert j's input DMAs: the m=0 gate/up head
            first (the first matmul needs only it), the activations, then
            the weight tails in arrival-deadline order, balanced across
            both HWDGE queues. Few instructions, long rows."""
            wgu_t = wgup.tile([128, KI, 2, KH, 128], BF16)
            nc.sync.dma_start(out=wgu_t[:, 0], in_=wgu[j, :, 0])
            xg_t = xgp.tile([128, KH, caps[j]], BF16)
            nc.scalar.dma_start(out=xg_t[:, 0:2, :], in_=xg[j][:, 0:2, :])
            nc.sync.dma_start(out=wgu_t[:, 1:3], in_=wgu[j, :, 1:3])
            nc.scalar.dma_start(out=xg_t[:, 2:, :], in_=xg[j][:, 2:, :])
            nc.sync.dma_start(out=wgu_t[:, 3:], in_=wgu[j, :, 3:])
            wd_t = wdp.tile([128, KH, KI, 128], BF16)
            nc.scalar.dma_start(out=wd_t, in_=wd[j])
            return wgu_t, xg_t, wd_t

        for rep in range(reps):
            if rep == 0:
                pend = issue_expert(0)
            for j in range(EPC):
                cap = caps[j]
                cts = _ctiles(cap)
                wgu_t, xg_t, wd_t = pend
                prefetched = False
                h_t = hp.tile([128, KI, cap], BF16)
                for m in range(KI):
                    for (c0, cn) in cts:
                        gp = ps_gu.tile([128, cn], F32, tag="gp")
                        for k in range(KH):
                            nc.tensor.matmul(gp, wgu_t[:, m, 0, k, :],
                                             xg_t[:, k, c0:c0 + cn],
                                             start=(k == 0), stop=(k == KH - 1))
                        up = ps_gu.tile([128, cn], F32, tag="up")
                        for k in range(KH):
                            nc.tensor.matmul(up, wgu_t[:, m, 1, k, :],
                                             xg_t[:, k, c0:c0 + cn],
                                             start=(k == 0), stop=(k == KH - 1))
                        sg = msc.tile([128, cn], F32, tag="sg")
                        nc.scalar.activation(sg, gp,
                                             mybir.ActivationFunctionType.Silu)
                        t1 = msc.tile([128, cn], F32, tag="t1")
                        nc.vector.tensor_mul(t1, sg, up)
                        nc.vector.tensor_mul(h_t[:, m, c0:c0 + cn], t1,
                                             cwb[j][:, c0:c0 + cn])
                    if not prefetched:
                        # software-pipeline the next expert's DMAs under the
                        # remaining gate/up compute (~20 us of cover)
                        prefetched = True
                        if j + 1 < EPC:
                            pend = issue_expert(j + 1)
                        elif rep + 1 < reps:
                            pend = issue_expert(0)
                tail = (j == EPC - 1 and rep + 1 == reps)
                yo_all = outp.tile([128, KH, cap], F32, tag="yo")
                for h in range(KH):
                    for ti, (c0, cn) in enumerate(cts):
                        yp = ps_d.tile([128, cn], F32, tag="yp")
                        for k in range(KI):
                            nc.tensor.matmul(yp, wd_t[:, h, k, :],
                                             h_t[:, k, c0:c0 + cn],
                                             start=(k == 0), stop=(k == KI - 1))
                        nc.vector.tensor_copy(yo_all[:, h, c0:c0 + cn], yp)
                        if tail:
                            # final expert: per-tile flush on the by-then-
                            # idle HWDGE queues — minimal drain tail
                            eng = nc.scalar if (h + ti) % 2 == 0 else nc.sync
                            eng.dma_start(out=yg[j][:, h, c0:c0 + cn],
                                          in_=yo_all[:, h, c0:c0 + cn])
                    if not tail:
                        # stream out per-h as each block completes (SWDGE
                        # keeps the HWDGE queues free for input reads)
                        nc.gpsimd.dma_start(out=yg[j][:, h:h + 1, :],
                                            in_=yo_all[:, h:h + 1, :])
    nc.finalize()
    return nc


def _get_router():
    if "router" not in _program_cache:
        _program_cache["router"] = build_router()
    return _program_cache["router"]


def _get_experts(caps):
    key = ("experts", tuple(caps))
    if key not in _program_cache:
        _program_cache[key] = build_experts(caps)
    return _program_cache[key]


def prep_router_inputs(x):
    xT = np.ascontiguousarray(x.T)
    return xT


def plan_experts(combine):
    """Load-balanced assignment: experts sorted by load descending; slot j
    takes ranks [8j, 8j+8), one per core; cap_j = max load in slot j
    (rounded up to 32). Returns per-expert index lists, assignment
    (core, slot) -> expert, and slot capacities."""
    loads = (combine > 0).sum(axis=0).astype(np.int64)
    order = np.argsort(-loads, kind="stable")
    assign = [[int(order[8 * j + c]) for j in range(EPC)]
              for c in range(NCORES)]
    caps = tuple(int(max(64, math.ceil(loads[order[8 * j]] / 8) * 8))
                 for j in range(EPC))
    idx = [np.nonzero(combine[:, e])[0] for e in range(E)]
    return idx, assign, caps


def prep_expert_weights(w_gate, w_up, w_down):
    """Per-expert bf16 tile-exact layouts (done once per kernel() call).
    gu: [E, 128(p), KI, 2, KH, 128(i)], d: [E, 128(p), KH, KI, 128(o)] —
    partition-major so each expert's weights are one long-row DMA."""
    gb = w_gate.astype(NPBF16).reshape(E, KI, 128, KH, 128)  # (e,m,i,k,p)
    ub = w_up.astype(NPBF16).reshape(E, KI, 128, KH, 128)
    gu = np.stack([gb, ub], axis=2)                          # (e,m,gu,i,k,p)
    gu = np.ascontiguousarray(gu.transpose(0, 5, 1, 2, 4, 3))  # (e,p,m,gu,k,i)
    db = w_down.astype(NPBF16).reshape(E, KH, 128, KI, 128)  # (e,h,o,k,p)
    db = np.ascontiguousarray(db.transpose(0, 4, 1, 3, 2))   # (e,p,h,k,o)
    return gu, db


def prep_expert_inputs(x, combine, idx, assign, caps, gu, db):
    """Build per-core in_maps with tile-exact layouts (contiguous DMA)."""
    xb = x.astype(NPBF16)
    in_maps = []
    for c in range(NCORES):
        m = {}
        sel = [assign[c][j] for j in range(EPC)]
        m["wgu"] = np.ascontiguousarray(gu[sel])   # [EPC,128,KI,2,KH,128]
        m["wd"] = np.ascontiguousarray(db[sel])    # [EPC,128,KH,KI,128]
        for j in range(EPC):
            e = sel[j]
            ii = idx[e]
            n = len(ii)
            cap = caps[j]
            xgj = np.zeros((128, KH, cap), NPBF16)
            cwj = np.zeros((1, cap), np.float32)
            if n:
                # [n, H] -> [n, KH, 128] -> [128, KH, n]
                xgj[:, :, :n] = xb[ii].reshape(n, KH, 128).transpose(2, 1, 0)
                cwj[0, :n] = combine[ii, e]
            m[f"xg{j}"] = xgj
            m[f"cw{j}"] = cwj
        in_maps.append(m)
    return in_maps


def kernel(hidden_states, w_router, w_gate, w_up, w_down):
    x = np.ascontiguousarray(np.asarray(hidden_states, np.float32)).reshape(T, H)
    w_gate = np.asarray(w_gate, np.float32)
    w_up = np.asarray(w_up, np.float32)
    w_down = np.asarray(w_down, np.float32)
    xT = prep_router_inputs(x)
    wrT = np.ascontiguousarray(np.asarray(w_router, np.float32).T)   # [H, E]

    # ---- Phase 1: router on device ----
    nc1 = _get_router()
    in_maps1 = [
        {"xTc": np.ascontiguousarray(xT[:, c * TPC:(c + 1) * TPC]), "wrT": wrT}
        for c in range(NCORES)
    ]
    r1 = run_bass_kernel_spmd(nc1, in_maps1, list(range(NCORES)))
    combine = np.concatenate(
        [r1.results[c]["comb"].reshape(TPC, E) for c in range(NCORES)], axis=0)

    # ---- Host: compaction (data movement only) ----
    idx, assign, caps = plan_experts(combine)
    gu, db = prep_expert_weights(w_gate, w_up, w_down)
    in_maps2 = prep_expert_inputs(x, combine, idx, assign, caps, gu, db)

    # ---- Phase 2: expert MLPs on device ----
    nc2 = _get_experts(caps)
    r2 = run_bass_kernel_spmd(nc2, in_maps2, list(range(NCORES)))

    # ---- Host: scatter-add (unique indices per expert) ----
    y = np.zeros((T, H), np.float32)
    for c in range(NCORES):
        for j in range(EPC):
            e = assign[c][j]
            ii = idx[e]
            n = len(ii)
            if n:
                # [128(p), KH(h), cap] -> [H, cap]: H index = h*128 + p
                ygj = r2.results[c][f"yg{j}"]
                yt = ygj.transpose(1, 0, 2).reshape(H, caps[j])
                y[ii] += yt[:, :n].T
    return y.reshape(B, S, H)


# revision 28
# speedup vs baseline: 1.3307x; 1.1649x over previous
"""MoE (MiMoV2 FlashMoE) Trainium2 kernel: expert-parallel over 8 NeuronCores.

Strategy:
  Phase 1 (device): router — logits = x @ w_router.T computed in exact fp32
    (selection must match the reference bit-for-bit; any top-4 flip costs
    ~14% output error), top-4 via iterative max/mask, combine weights =
    sigmoid(logit) normalized over the selected 4. Each core handles
    T/8 = 512 tokens. Output: dense combine matrix [T, E].
  Host: compaction — per-expert token index lists (data movement only),
    load-balanced expert->(core,slot) assignment: experts sorted by load
    descending, slot j takes ranks [8j, 8j+8) one per core, slot capacity
    = max load in the slot (rounded up to 32). This cuts padded columns
    ~16% vs a global fixed capacity.
  Phase 2 (device): experts — one expert per (core, slot). All matmuls in
    bf16 (same 1 cycle/row PE rate as f32r on TRN2, half the HBM/SBUF
    traffic; output rel err ~3e-3 vs the 2e-2 budget). G = Wg x, U = Wu x
    accumulate in fp32 PSUM; H = silu(G)*U*combine in fp32, cast to bf16;
    Y = Wd H in fp32 out.
  Host: scatter-add per-expert outputs into y [T, H].
"""
import math
import numpy as np
import ml_dtypes
from contextlib import ExitStack

import concourse.bass as bass
import concourse.mybir as mybir
import concourse.tile as tile
from concourse import bacc
from concourse.bass_utils import run_bass_kernel_spmd

F32 = mybir.dt.float32
F32R = mybir.dt.float32r
BF16 = mybir.dt.bfloat16
NPBF16 = ml_dtypes.bfloat16

# Problem shapes (hardcoded per contract)
E = 32          # experts
TOPK = 4
H = 1024        # hidden
I = 768         # intermediate
B, S = 2, 2048
T = B * S       # 4096 tokens
NCORES = 8
EPC = E // NCORES    # expert slots per core = 4
TPC = T // NCORES    # router tokens per core = 512
KH = H // 128        # 8 contraction chunks over H
KI = I // 128        # 6 contraction chunks over I

_program_cache = {}


def _ctiles(C):
    """Split C into near-equal tiles, each <= 512 (PSUM bank)."""
    n = max(1, math.ceil(C / 512))
    base = C // n
    rem = C - base * n
    sizes = [base + (1 if i < rem else 0) for i in range(n)]
    out, off = [], 0
    for s in sizes:
        out.append((off, s))
        off += s
    return out


def build_router(reps=1):
    """Per-core: logits computed token-major directly — per 128-token tile,
    stationary = x chunk [128(H), 128(tok)], moving = w_router chunk
    [128(H), E], accumulating logits [128(tok), E] over 8 H-chunks in PSUM.
    Contraction stays on the partition dim, so the fp32 numerics are
    identical to the transposed layout, with no DVE transposes. Then a
    batched top-4 + combine-weight computation on a single [128, 4, E]
    tile. Selection compares exact fp32 logits."""
    nc = bacc.Bacc()
    NT = TPC // 128  # 4 token tiles
    xTc = nc.dram_tensor("xTc", [KH, 128, TPC], F32, kind="ExternalInput")
    wrT = nc.dram_tensor("wrT", [KH, 128, E], F32, kind="ExternalInput")
    comb_out = nc.dram_tensor("comb", [NT, 128, E], F32, kind="ExternalOutput")
    with ExitStack() as ctx:
        tc = ctx.enter_context(tile.TileContext(nc))
        sb = ctx.enter_context(tc.tile_pool(name="sb", bufs=1))
        work = ctx.enter_context(tc.tile_pool(name="work", bufs=2))
        ps = ctx.enter_context(tc.tile_pool(name="ps", bufs=2, space="PSUM"))

        # preload the sigmoid table while the inputs stream in
        dum = sb.tile([128, 1], F32)
        nc.gpsimd.memset(dum, 0.0)
        nc.scalar.activation(dum, dum, mybir.ActivationFunctionType.Sigmoid)

        # transposed-AP DMAs: 5 HWDGE passes instead of 16; xr split per
        # token tile so the first matmuls overlap the remaining transfer
        xr = sb.tile([128, KH, TPC], F32)
        wr = sb.tile([128, KH, E], F32)
        nc.scalar.dma_start(out=wr, in_=wrT[:, :, :].transpose([1, 0, 2]))
        for t in range(NT):
            eng = nc.sync if t % 2 == 0 else nc.scalar
            eng.dma_start(
                out=xr[:, :, t * 128:(t + 1) * 128],
                in_=xTc[:, :, t * 128:(t + 1) * 128].transpose([1, 0, 2]))

        for _ in range(reps):
            lt_p = ps.tile([128, NT, E], F32)   # logits, token-major
            for t in range(NT):
                for k in range(KH):
                    nc.tensor.matmul(lt_p[:, t, :],
                                     xr[:, k, t * 128:(t + 1) * 128],
                                     wr[:, k, :],
                                     start=(k == 0), stop=(k == KH - 1))
            lt = work.tile([128, NT, E], F32)
            nc.vector.tensor_copy(lt, lt_p)
            # batched top-4: find 4th max per token via iterative masking
            cur = work.tile([128, NT, E], F32)
            nc.vector.tensor_copy(cur, lt)
            m = work.tile([128, NT, 1], F32)
            ge = work.tile([128, NT, E], F32)
            for _k in range(TOPK - 1):
                nc.vector.reduce_max(m, cur, axis=mybir.AxisListType.X)
                nc.vector.tensor_tensor(ge, cur, m.broadcast_to((128, NT, E)),
                                        op=mybir.AluOpType.is_ge)
                nc.vector.scalar_tensor_tensor(cur, ge, -1e30, cur,
                                               op0=mybir.AluOpType.mult,
                                               op1=mybir.AluOpType.add)
            nc.vector.reduce_max(m, cur, axis=mybir.AxisListType.X)
            # sel = (logits >= 4th max), combine = sel*sigmoid normalized
            sel = work.tile([128, NT, E], F32)
            nc.vector.tensor_tensor(sel, lt, m.broadcast_to((128, NT, E)),
                                    op=mybir.AluOpType.is_ge)
            sig = work.tile([128, NT, E], F32)
            nc.scalar.activation(sig, lt, mybir.ActivationFunctionType.Sigmoid)
            wsel = work.tile([128, NT, E], F32)
            nc.vector.tensor_mul(wsel, sel, sig)
            ssum = work.tile([128, NT, 1], F32)
            nc.vector.reduce_sum(ssum, wsel, axis=mybir.AxisListType.X)
            nc.vector.tensor_scalar_add(ssum, ssum, 1e-20)
            rsum = work.tile([128, NT, 1], F32)
            nc.vector.reciprocal(rsum, ssum)
            ct = work.tile([128, NT, E], F32)
            nc.vector.tensor_tensor(ct, wsel, rsum.broadcast_to((128, NT, E)),
                                    op=mybir.AluOpType.mult)
            nc.sync.dma_start(out=comb_out[:, :, :].transpose([1, 0, 2]), in_=ct)
    nc.finalize()
    return nc


def build_experts(caps, reps=1):
    """Expert MLP kernel, one expert per slot j with capacity caps[j].
    Per-core inputs (pre-laid-out for SBUF tiles, all bf16 except cw).
    Weight tensors are partition-major so each expert's weights move in
    ONE long-row DMA (128 descriptors) instead of 12/8 short-row DMAs —
    DMA-issue SEQ time on the HWDGE queues is proportional to descriptor
    count and was the limiting resource:
      xg{j} [128, KH, caps[j]]          bf16  xg[p,k,c] = x[tok_c, k*128+p]
      wgu   [EPC, 128, KI, 2, KH, 128]  bf16  [j,p,m,0,k,i]=w_gate[e,m*128+i,k*128+p]
      wd    [EPC, 128, KH, KI, 128]     bf16  wd[j,p,h,k,o]=w_down[e,h*128+o,k*128+p]
      cw{j} [1, caps[j]]                f32   combine weights (0 on padding)
    Output: yg{j} [128, KH, caps[j]] f32, yg[p,h,c] = y^T[h*128+p, c]
    (combine-weighted, transposed)."""
    caps = tuple(caps)
    nc = bacc.Bacc()
    xg = [nc.dram_tensor(f"xg{j}", [128, KH, caps[j]], BF16,
                         kind="ExternalInput") for j in range(EPC)]
    wgu = nc.dram_tensor("wgu", [EPC, 128, KI, 2, KH, 128], BF16,
                         kind="ExternalInput")
    wd = nc.dram_tensor("wd", [EPC, 128, KH, KI, 128], BF16,
                        kind="ExternalInput")
    cw = [nc.dram_tensor(f"cw{j}", [1, caps[j]], F32, kind="ExternalInput")
          for j in range(EPC)]
    yg = [nc.dram_tensor(f"yg{j}", [128, KH, caps[j]], F32,
                         kind="ExternalOutput") for j in range(EPC)]
    warm_out = nc.dram_tensor("warm", [128, 1], F32, kind="ExternalOutput")

    with ExitStack() as ctx:
        tc = ctx.enter_context(tile.TileContext(nc))
        cwp = ctx.enter_context(tc.tile_pool(name="cwp", bufs=1))
        xgp = ctx.enter_context(tc.tile_pool(name="xgp", bufs=2))
        wgup = ctx.enter_context(tc.tile_pool(name="wgup", bufs=2))
        wdp = ctx.enter_context(tc.tile_pool(name="wdp", bufs=2))
        hp = ctx.enter_context(tc.tile_pool(name="hp", bufs=2))
        msc = ctx.enter_context(tc.tile_pool(name="msc", bufs=4))
        outp = ctx.enter_context(tc.tile_pool(name="outp", bufs=2))
        ps_gu = ctx.enter_context(tc.tile_pool(name="ps_gu", bufs=2, space="PSUM"))
        ps_d = ctx.enter_context(tc.tile_pool(name="ps_d", bufs=2, space="PSUM"))

        # PE warm-up: keep TensorE busy while the first weight/activation
        # DMAs land, so the HAM clock-gate releases (1.2 -> 2.4 GHz) before
        # real matmuls start. Results are dumped to a debug output. Issued
        # before everything else so nothing delays the first warm matmul.
        wtile = cwp.tile([128, 512], F32R, tag="warm")
        nc.gpsimd.memset(wtile.bitcast(F32), 0.0)
        wps = ps_d.tile([128, 512], F32, tag="warmp")
        NWARM = 20
        for wi in range(NWARM):
            nc.tensor.matmul(wps, wtile[:, :128], wtile,
                             start=(wi == 0), stop=(wi == NWARM - 1))

        cwb = []
        for j in range(EPC):
            cwt = cwp.tile([128, caps[j]], F32, tag=f"cw{j}")
            nc.gpsimd.dma_start(out=cwt,
                                in_=cw[j][0:1, :].partition_broadcast(128))
            cwb.append(cwt)

        # warm-out readback has no deadline; issue it after the cw loads
        wres = cwp.tile([128, 1], F32, tag="warmres")
        nc.vector.tensor_copy(wres, wps[:, 0:1])
        nc.gpsimd.dma_start(out=warm_out[:], in_=wres)

        def issue_expert(j):
            """Enqueue ALL of expert j's input DMAs: the m=0 gate/up head
            first (the first matmul needs only it), the activations, then
            the weight tails in arrival-deadline order, balanced across
            both HWDGE queues. Few instructions, long rows."""
            wgu_t = wgup.tile([128, KI, 2, KH, 128], BF16)
            nc.sync.dma_start(out=wgu_t[:, 0], in_=wgu[j, :, 0])
            xg_t = xgp.tile([128, KH, caps[j]], BF16)
            nc.scalar.dma_start(out=xg_t[:, 0:2, :], in_=xg[j][:, 0:2, :])
            nc.sync.dma_start(out=wgu_t[:, 1:3], in_=wgu[j, :, 1:3])
            nc.scalar.dma_start(out=xg_t[:, 2:, :], in_=xg[j][:, 2:, :])
            nc.sync.dma_start(out=wgu_t[:, 3:], in_=wgu[j, :, 3:])
            wd_t = wdp.tile([128, KH, KI, 128], BF16)
            nc.scalar.dma_start(out=wd_t, in_=wd[j])
            return wgu_t, xg_t, wd_t

        for rep in range(reps):
            if rep == 0:
                pend = issue_expert(0)
            for j in range(EPC):
                cap = caps[j]
                cts = _ctiles(cap)
                wgu_t, xg_t, wd_t = pend
                prefetched = False
                h_t = hp.tile([128, KI, cap], BF16)
                for m in range(KI):
                    for (c0, cn) in cts:
                        gp = ps_gu.tile([128, cn], F32, tag="gp")
                        for k in range(KH):
                            nc.tensor.matmul(gp, wgu_t[:, m, 0, k, :],
                                             xg_t[:, k, c0:c0 + cn],
                                             start=(k == 0), stop=(k == KH - 1))
                        up = ps_gu.tile([128, cn], F32, tag="up")
                        for k in range(KH):
                            nc.tensor.matmul(up, wgu_t[:, m, 1, k, :],
                                             xg_t[:, k, c0:c0 + cn],
                                             start=(k == 0), stop=(k == KH - 1))
                        sg = msc.tile([128, cn], F32, tag="sg")
                        nc.scalar.activation(sg, gp,
                                             mybir.ActivationFunctionType.Silu)
                        t1 = msc.tile([128, cn], F32, tag="t1")
                        nc.vector.tensor_mul(t1, sg, up)
                        nc.vector.tensor_mul(h_t[:, m, c0:c0 + cn], t1,
                                             cwb[j][:, c0:c0 + cn])
                    if not prefetched:
                        # software-pipeline the next expert's DMAs under the
                        # remaining gate/up compute (~20 us of cover)
                        prefetched = True
                        if j + 1 < EPC:
                            pend = issue_expert(j + 1)
                        elif rep + 1 < reps:
                            pend = issue_expert(0)
                tail = (j == EPC - 1 and rep + 1 == reps)
                yo_all = outp.tile([128, KH, cap], F32, tag="yo")
                for h in range(KH):
                    for ti, (c0, cn) in enumerate(cts):
                        yp = ps_d.tile([128, cn], F32, tag="yp")
                        for k in range(KI):
                            nc.tensor.matmul(yp, wd_t[:, h, k, :],
                                             h_t[:, k, c0:c0 + cn],
                                             start=(k == 0), stop=(k == KI - 1))
                        nc.vector.tensor_copy(yo_all[:, h, c0:c0 + cn], yp)
                        if tail:
                            # final expert: per-tile flush on the by-then-
                            # idle HWDGE queues — minimal drain tail
                            eng = nc.scalar if (h + ti) % 2 == 0 else nc.sync
                            eng.dma_start(out=yg[j][:, h, c0:c0 + cn],
                                          in_=yo_all[:, h, c0:c0 + cn])
                    if not tail:
                        # stream out per-h as each block completes (SWDGE
                        # keeps the HWDGE queues free for input reads)
                        nc.gpsimd.dma_start(out=yg[j][:, h:h + 1, :],
                                            in_=yo_all[:, h:h + 1, :])
    nc.finalize()
    return nc


def _get_router():
    if "router" not in _program_cache:
        _program_cache["router"] = build_router()
    return _program_cache["router"]


def _get_experts(caps):
    key = ("experts", tuple(caps))
    if key not in _program_cache:
        _program_cache[key] = build_experts(caps)
    return _program_cache[key]


def prep_router_inputs(x):
    xT = np.ascontiguousarray(x.T)
    return xT


def plan_experts(combine):
    """Load-balanced assignment: experts sorted by load descending; slot j
    takes ranks [8j, 8j+8), one per core; cap_j = max load in slot j
    (rounded up to 32). Returns per-expert index lists, assignment
    (core, slot) -> expert, and slot capacities."""
    loads = (combine > 0).sum(axis=0).astype(np.int64)
    order = np.argsort(-loads, kind="stable")
    assign = [[int(order[8 * j + c]) for j in range(EPC)]
              for c in range(NCORES)]
    caps = tuple(int(max(64, math.ceil(loads[order[8 * j]] / 8) * 8))
                 for j in range(EPC))
    idx = [np.nonzero(combine[:, e])[0] for e in range(E)]
    return idx, assign, caps


def prep_expert_weights(w_gate, w_up, w_down):
    """Per-expert bf16 tile-exact layouts (done once per kernel() call).
    gu: [E, 128(p), KI, 2, KH, 128(i)], d: [E, 128(p), KH, KI, 128(o)] —
    partition-major so each expert's weights are one long-row DMA."""
    gb = w_gate.astype(NPBF16).reshape(E, KI, 128, KH, 128)  # (e,m,i,k,p)
    ub = w_up.astype(NPBF16).reshape(E, KI, 128, KH, 128)
    gu = np.stack([gb, ub], axis=2)                          # (e,m,gu,i,k,p)
    gu = np.ascontiguousarray(gu.transpose(0, 5, 1, 2, 4, 3))  # (e,p,m,gu,k,i)
    db = w_down.astype(NPBF16).reshape(E, KH, 128, KI, 128)  # (e,h,o,k,p)
    db = np.ascontiguousarray(db.transpose(0, 4, 1, 3, 2))   # (e,p,h,k,o)
    return gu, db


def prep_expert_inputs(x, combine, idx, assign, caps, gu, db):
    """Build per-core in_maps with tile-exact layouts (contiguous DMA)."""
    xb = x.astype(NPBF16)
    in_maps = []
    for c in range(NCORES):
        m = {}
        sel = [assign[c][j] for j in range(EPC)]
        m["wgu"] = np.ascontiguousarray(gu[sel])   # [EPC,128,KI,2,KH,128]
        m["wd"] = np.ascontiguousarray(db[sel])    # [EPC,128,KH,KI,128]
        for j in range(EPC):
            e = sel[j]
            ii = idx[e]
            n = len(ii)
            cap = caps[j]
            xgj = np.zeros((128, KH, cap), NPBF16)
            cwj = np.zeros((1, cap), np.float32)
            if n:
                # [n, H] -> [n, KH, 128] -> [128, KH, n]
                xgj[:, :, :n] = xb[ii].reshape(n, KH, 128).transpose(2, 1, 0)
                cwj[0, :n] = combine[ii, e]
            m[f"xg{j}"] = xgj
            m[f"cw{j}"] = cwj
        in_maps.append(m)
    return in_maps


def kernel(hidden_states, w_router, w_gate, w_up, w_down):
    x = np.ascontiguousarray(np.asarray(hidden_states, np.float32)).reshape(T, H)
    w_gate = np.asarray(w_gate, np.float32)
    w_up = np.asarray(w_up, np.float32)
    w_down = np.asarray(w_down, np.float32)
    xT = prep_router_inputs(x)
    wrT = np.ascontiguousarray(np.asarray(w_router, np.float32).T)   # [H, E]

    # ---- Phase 1: router on device ----
    nc1 = _get_router()
    wrT3 = wrT.reshape(KH, 128, E)
    in_maps1 = [
        {"xTc": np.ascontiguousarray(
            xT[:, c * TPC:(c + 1) * TPC]).reshape(KH, 128, TPC),
         "wrT": wrT3}
        for c in range(NCORES)
    ]
    r1 = run_bass_kernel_spmd(nc1, in_maps1, list(range(NCORES)))
    combine = np.concatenate(
        [r1.results[c]["comb"].reshape(TPC, E) for c in range(NCORES)], axis=0)

    # ---- Host: compaction (data movement only) ----
    idx, assign, caps = plan_experts(combine)
    gu, db = prep_expert_weights(w_gate, w_up, w_down)
    in_maps2 = prep_expert_inputs(x, combine, idx, assign, caps, gu, db)

    # ---- Phase 2: expert MLPs on device ----
    nc2 = _get_experts(caps)
    r2 = run_bass_kernel_spmd(nc2, in_maps2, list(range(NCORES)))

    # ---- Host: scatter-add (unique indices per expert) ----
    y = np.zeros((T, H), np.float32)
    for c in range(NCORES):
        for j in range(EPC):
            e = assign[c][j]
            ii = idx[e]
            n = len(ii)
            if n:
                # [128(p), KH(h), cap] -> [H, cap]: H index = h*128 + p
                ygj = r2.results[c][f"yg{j}"]
                yt = ygj.transpose(1, 0, 2).reshape(H, caps[j])
                y[ii] += yt[:, :n].T
    return y.reshape(B, S, H)
